# revision 1
# baseline (speedup 1.0000x reference)
"""MultiHeadAttention Trainium2 kernel (8-core SPMD).

Problem: B=2, T=2048, C=1024, H=16 heads, D=64.
  out = softmax((q Wq^T + bq)(k Wk^T + bk)^T / sqrt(D)) (v Wv^T + bv) Wo^T + bo

Sharding: core c -> (batch b = c // 4, head-group g = c % 4).  Each core
computes 4 heads (a 256-wide slice of the projection space) of one batch
element, including its partial contribution to the row-sharded output
projection.  The host sums the 4 partial outputs per batch and adds bo
(bo itself is folded on-device into the g==0 core's partial).

Per-core dataflow (all matmuls in float32r = tf32, fp32 accumulate):
  - PE-transpose q/k/v tiles to get channel-major activations (contraction
    over C needs C on partitions).
  - QT/KT/VT = W_s @ x^T  ([256, T] channel-major), bias folded in.
  - V_nat[k, d] from VT via PE transposes (needed as AV stationary operand).
  - S^T[k, q] = K_h Q_h^T per head (scores transposed -> no P transpose
    before AV); exp via ScalarE with scale=1/8 folded in; no max
    subtraction (|scores/8| ~ 2 for these inputs, exp is safe in fp32).
  - Row sums via ones-matmul (32 replicated rows per head), softmax
    normalization deferred to the [256, T] attention output.
  - partial^T[co, t] = Wo_s^T.T @ O^T accumulated over the 256 head dims.
"""

import numpy as np

B, T, C, H, D = 2, 2048, 1024, 16, 64
NCORES = 8
GROUPS = 4              # head-groups == cores per batch element
HG = H // GROUPS        # heads per core
DS = HG * D             # per-core projection slice width (256)
TCH = 512               # token chunk (psum bank = 512 fp32)
NTCH = T // TCH         # 4
NCC = C // 128          # 8 contraction chunks
NKT = T // 128          # 16 key tiles
SCALE = float(D) ** -0.5

_NC_CACHE = None

# timing probes: 0=full, 1=stage A only, 2=A+scores+exp, 3=A+B w/o sums,
# 4=full w/o out-proj
PROBE = 0
# dtype of the attention-probability path (es tiles + V-natural weights):
# "f32r" (tf32, best accuracy) or "bf16" (halves PE weight-load time)
AV_DT = "f32r"


def _emit(ctx, tc, io):
    import concourse.bass as bass
    from concourse import mybir

    nc = tc.nc
    f32 = mybir.dt.float32
    f32r = mybir.dt.float32r
    bf16 = mybir.dt.bfloat16
    EXP = mybir.ActivationFunctionType.Exp

    persist = ctx.enter_context(tc.tile_pool(name="persist", bufs=1))

    def ptile(tag, shape, dt=f32):
        return persist.tile(shape, dt, tag=tag, name=tag)

    # --- persistent SBUF tensors ---------------------------------------
    ident = ptile("ident", [128, 128])
    nc.sync.dma_start(ident[:], io["ident"][:, :])
    ones_f = ptile("ones_f", [128, 64])
    nc.vector.memset(ones_f[:], 1.0)

    wsb = {}
    for name in ("wqt", "wkt", "wvt"):
        tiles = []
        for c in range(NCC):
            t_ = ptile(f"{name}{c}", [128, DS], f32r)
            nc.scalar.dma_start(t_[:], io[name][c * 128:(c + 1) * 128, :])
            tiles.append(t_)
        wsb[name] = tiles
    wot = []
    for dc in range(2):
        t_ = ptile(f"wot{dc}", [128, C], f32r)
        nc.scalar.dma_start(t_[:], io["wot"][dc * 128:(dc + 1) * 128, :])
        wot.append(t_)

    bias = {}
    for name, width in (("bqs", 2), ("bks", 2), ("bvs", 2), ("bos", 8)):
        t_ = ptile(name, [128, width])
        nc.scalar.dma_start(
            t_[:], io[name].rearrange("(a p) o -> p (a o)", p=128))
        bias[name] = t_

    QT = [ptile(f"qt{i}", [128, T], f32r) for i in range(2)]
    KT = [ptile(f"kt{i}", [128, T], f32r) for i in range(2)]
    # V natural [k, head*(64 V + 64 ones)]: the ones columns make each
    # head's AV matmul also produce the softmax denominator (replicated
    # across psum rows 64-127)
    av_dt = f32r if AV_DT == "f32r" else bf16
    VN = [ptile(f"vn{i}", [128, 4 * 128], av_dt) for i in range(NKT)]

    probe = PROBE

    # --- stage A: transpose + project q, k, v --------------------------
    with tc.tile_pool(name="vt", bufs=1) as vtp, \
         tc.tile_pool(name="nat", bufs=4) as natp, \
         tc.tile_pool(name="xtsb", bufs=10) as xtsbp, \
         tc.tile_pool(name="xtps", bufs=3, space="PSUM") as xtps, \
         tc.tile_pool(name="projps", bufs=3, space="PSUM") as projps:

        VT = [vtp.tile([128, T], f32r, tag=f"vt{i}", name="vt")
              for i in range(2)]

        def stage_a(xname, wname, bname, XT, tci, ring):
            nat = natp.tile([128, 4 * C], f32, tag="nat", name="nat")
            src_ap = io[xname][tci * TCH:(tci + 1) * TCH, :].rearrange(
                "(j p) c -> p j c", p=128)
            dst_ap = nat[:].rearrange("p (j c) -> p j c", j=4)
            eng = nc.sync if ring % 2 == 0 else nc.scalar
            eng.dma_start(dst_ap, src_ap)
            xts = []
            for c in range(NCC):
                ps = xtps.tile([128, TCH], f32, tag="xt", name="xtps")
                for j in range(4):
                    nc.tensor.matmul(
                        ps[:, j * 128:(j + 1) * 128],
                        lhsT=nat[:, j * C + c * 128:j * C + (c + 1) * 128],
                        rhs=ident[:],
                        is_transpose=True,
                        start=(j == 0), stop=(j == 3))
                xt = xtsbp.tile([128, TCH], f32r, tag="xt", name="xtsb")
                if c % 2 == 0:
                    nc.scalar.copy(xt[:], ps[:])
                else:
                    nc.vector.tensor_copy(xt[:], ps[:])
                xts.append(xt)
            for co in range(2):
                pj = projps.tile([128, TCH], f32, tag="proj", name="proj")
                for c in range(NCC):
                    nc.tensor.matmul(
                        pj[:],
                        lhsT=wsb[wname][c][:, co * 128:(co + 1) * 128],
                        rhs=xts[c][:],
                        start=(c == 0), stop=(c == NCC - 1))
                nc.vector.tensor_scalar_add(
                    XT[co][:, tci * TCH:(tci + 1) * TCH],
                    pj[:], bias[bname][:, co:co + 1])

        # interleave k/v/q chunks for more independent PE work in flight
        for tci in range(NTCH):
            stage_a("xk", "wkt", "bks", KT, tci, 2 * tci)
            stage_a("xv", "wvt", "bvs", VT, tci, 2 * tci + 1)
        # V natural tiles from VT via PE transposes (before q so the q
        # DMAs prefetch under this PE work)
        for tp in range(NKT // 2):
            ps = xtps.tile([128, TCH], f32, tag="xt", name="xtps")
            for u in range(2):
                tt = 2 * tp + u
                for dc in range(2):
                    q_ = 2 * u + dc
                    nc.tensor.matmul(
                        ps[:, q_ * 128:(q_ + 1) * 128],
                        lhsT=VT[dc][:, tt * 128:(tt + 1) * 128].bitcast(f32),
                        rhs=ident[:],
                        is_transpose=True,
                        start=(q_ == 0), stop=(q_ == 3))
            for u in range(2):
                vn = VN[2 * tp + u]
                src3 = ps[:, u * DS:(u + 1) * DS].rearrange(
                    "p (h d) -> p h d", h=4)
                dst3 = vn[:].rearrange("p (h c) -> p h c", h=4)[:, :, 0:64]
                eng_c = nc.scalar.copy if u == 0 else nc.vector.tensor_copy
                eng_c(dst3, src3)
                dst1 = vn[:].rearrange("p (h c) -> p h c", h=4)[:, :, 64:128]
                for h in range(4):
                    nc.vector.tensor_copy(dst1[:, h, :], ones_f[:])

        for tci in range(NTCH):
            stage_a("xq", "wqt", "bqs", QT, tci, tci)

    # --- stage B/C: attention + output projection ----------------------
    # S pool: bufs=3 of [128, 1024] (6 banks) -> the scores->exp->AV chain
    # pipelines 3 deep; per-head accumulators for ONE pair at a time
    # (2 banks).  Head pairs run as two passes per q-chunk.  The
    # out-projection borrows S slots.
    with tc.tile_pool(name="sps", bufs=3, space="PSUM") as sps, \
         tc.tile_pool(name="otps", bufs=2, space="PSUM") as otps, \
         tc.tile_pool(name="expsb", bufs=4) as expsb, \
         tc.tile_pool(name="otsb", bufs=4) as otsbp, \
         tc.tile_pool(name="recsb", bufs=4) as recp, \
         tc.tile_pool(name="outsb", bufs=3) as outsbp:

        for qc in range(NTCH):
            if probe == 1:
                break
            qcols = slice(qc * TCH, (qc + 1) * TCH)
            ot_sb = []
            for pr in range(2):
                otp = [otps.tile([128, TCH], f32, tag="ot", name="ot")
                       for _ in range(2)]
                for g in range(NKT // 2):
                    first = (g == 0)
                    last = (g == NKT // 2 - 1)
                    for hh in range(2):
                        h = pr * 2 + hh
                        rows = slice(hh * 64, (hh + 1) * 64)
                        S = sps.tile([128, 2 * TCH], f32, tag="s", name="s")
                        for j in range(2):
                            kt = 2 * g + j
                            nc.tensor.matmul(
                                S[:, j * TCH:(j + 1) * TCH],
                                lhsT=KT[pr][rows, kt * 128:(kt + 1) * 128],
                                rhs=QT[pr][rows, qcols],
                                start=True, stop=True)
                        es = expsb.tile([128, 2 * TCH], av_dt, tag="es",
                                        name="es")
                        nc.scalar.activation(es[:], S[:], EXP, scale=SCALE)
                        if probe == 2:
                            continue
                        for j in range(2):
                            kt = 2 * g + j
                            nc.tensor.matmul(
                                otp[hh][:, :],
                                lhsT=VN[kt][:, h * 128:(h + 1) * 128],
                                rhs=es[:, j * TCH:(j + 1) * TCH],
                                start=(first and j == 0),
                                stop=(last and j == 1))
                if probe == 2:
                    continue
                # normalize: psum rows 64-127 hold the denominator
                osb = otsbp.tile([128, TCH], f32r, tag="otsb", name="otsb")
                for hh in range(2):
                    rec = recp.tile([64, TCH], f32, tag="rec", name="rec")
                    nc.vector.reciprocal(rec[:], otp[hh][64:128, :])
                    nc.vector.tensor_mul(
                        osb[hh * 64:(hh + 1) * 64, :],
                        otp[hh][0:64, :], rec[:])
                ot_sb.append(osb)
            if probe == 2:
                continue
            if probe == 4:
                continue
            for ct in range(NCC):
                pp = sps.tile([128, TCH], f32, tag="s", name="prj")
                for dc in range(2):
                    nc.tensor.matmul(
                        pp[:],
                        lhsT=wot[dc][:, ct * 128:(ct + 1) * 128],
                        rhs=ot_sb[dc][:],
                        start=(dc == 0), stop=(dc == 1))
                ob = outsbp.tile([128, TCH], f32, tag="ob", name="ob")
                nc.vector.tensor_scalar_add(
                    ob[:], pp[:], bias["bos"][:, ct:ct + 1])
                nc.sync.dma_start(
                    io["out_t"][ct * 128:(ct + 1) * 128, qcols], ob[:])


def build_nc(reps=1):
    from contextlib import ExitStack

    import concourse.tile as tile
    from concourse import bacc, mybir

    f32 = mybir.dt.float32
    nc = bacc.Bacc("TRN2", target_bir_lowering=False, debug=False,
                   num_devices=NCORES)
    io = {}
    for name in ("xq", "xk", "xv"):
        io[name] = nc.dram_tensor(name, [T, C], f32, kind="ExternalInput").ap()
    f32r = mybir.dt.float32r
    for name in ("wqt", "wkt", "wvt"):
        io[name] = nc.dram_tensor(name, [C, DS], f32r,
                                  kind="ExternalInput").ap()
    io["wot"] = nc.dram_tensor("wot", [DS, C], f32r, kind="ExternalInput").ap()
    for name in ("bqs", "bks", "bvs"):
        io[name] = nc.dram_tensor(name, [DS, 1], f32, kind="ExternalInput").ap()
    io["bos"] = nc.dram_tensor("bos", [C, 1], f32, kind="ExternalInput").ap()
    io["ident"] = nc.dram_tensor("ident", [128, 128], f32,
                                 kind="ExternalInput").ap()
    io["out_t"] = nc.dram_tensor("out_t", [C, T], f32,
                                 kind="ExternalOutput").ap()

    with tile.TileContext(nc) as tc:
        if reps == 1:
            with ExitStack() as ctx:
                _emit(ctx, tc, io)
        else:
            with tc.For_i(0, reps, 1):
                with ExitStack() as ctx:
                    _emit(ctx, tc, io)
    nc.compile()
    return nc


def get_nc():
    global _NC_CACHE
    if _NC_CACHE is None:
        _NC_CACHE = build_nc()
    return _NC_CACHE


def tf32_round(x):
    """Round fp32 to tf32 (10-bit mantissa, round-to-nearest-even)."""
    u = np.ascontiguousarray(x, np.float32).view(np.uint32)
    u = (u + 0xFFF + ((u >> 13) & 1)) & np.uint32(0xFFFFE000)
    return u.view(np.float32)


def make_in_maps(q, k, v, Wq, bq, Wk, bk, Wv, bv, Wo, bo):
    q, k, v = (np.asarray(x, np.float32) for x in (q, k, v))
    Wq, Wk, Wv, Wo = (np.asarray(x, np.float32) for x in (Wq, Wk, Wv, Wo))
    bq, bk, bv, bo = (np.asarray(x, np.float32) for x in (bq, bk, bv, bo))
    ident = np.eye(128, dtype=np.float32)
    zeros_c = np.zeros((C, 1), np.float32)
    in_maps = []
    for core in range(NCORES):
        b, g = divmod(core, GROUPS)
        sl = slice(g * DS, (g + 1) * DS)
        in_maps.append({
            "xq": np.ascontiguousarray(q[b]),
            "xk": np.ascontiguousarray(k[b]),
            "xv": np.ascontiguousarray(v[b]),
            "wqt": tf32_round(np.ascontiguousarray(Wq[sl, :].T)),
            "wkt": tf32_round(np.ascontiguousarray(Wk[sl, :].T)),
            "wvt": tf32_round(np.ascontiguousarray(Wv[sl, :].T)),
            "wot": tf32_round(np.ascontiguousarray(Wo[:, sl].T)),
            "bqs": np.ascontiguousarray(bq[sl].reshape(DS, 1)),
            "bks": np.ascontiguousarray(bk[sl].reshape(DS, 1)),
            "bvs": np.ascontiguousarray(bv[sl].reshape(DS, 1)),
            "bos": (np.ascontiguousarray(bo.reshape(C, 1))
                    if g == 0 else zeros_c),
            "ident": ident,
        })
    return in_maps


def combine(results):
    out = np.zeros((B, T, C), np.float32)
    for core in range(NCORES):
        b, _ = divmod(core, GROUPS)
        out[b] += results[core]["out_t"].T
    return out


def kernel(q, k, v, Wq, bq, Wk, bk, Wv, bv, Wo, bo):
    from concourse.bass_utils import run_bass_kernel_spmd

    nc = get_nc()
    in_maps = make_in_maps(q, k, v, Wq, bq, Wk, bk, Wv, bv, Wo, bo)
    res = run_bass_kernel_spmd(nc, in_maps, core_ids=list(range(NCORES)))
    return combine(res.results)



# revision 32
# speedup vs baseline: 1.6635x; 1.6635x over previous
"""MultiHeadAttention Trainium2 kernel (8-core SPMD), final.

Problem: B=2, T=2048, C=1024, H=16 heads, D=64.
  out = softmax((q Wq^T + bq)(k Wk^T + bk)^T / sqrt(D)) (v Wv^T + bv) Wo^T + bo

Sharding: core c -> (batch b = c // 4, head-group g = c % 4).  Each core
computes 4 heads (a 256-wide slice of the projection space) of one batch
element, including its partial contribution to the row-sharded output
projection.  The host sums the 4 partial outputs per batch; bo and the
linear bv contribution (Wo[:, sl] @ bv[sl]) fold into a per-core output
bias added on device.

Design notes (each validated against HW NTFF profiles; 507us -> ~302us):
  - bf16 end to end (tolerance 2e-2; lands ~4.7e-3).  fp8 is NOT usable:
    random-sign contractions keep per-term relative error, so fp8e4m3
    anywhere in the mainline costs ~6% output error.
  - V is projected directly into natural [token, dim] orientation by
    swapping matmul operands (v^T tile stationary, Wv^T moving), so no
    separate V transpose pass; each V tile carries 64 ones columns per
    head so the AV matmul also emits the softmax denominators for free.
  - ScalarE runs (almost) only the 142us of exp activations - the hard
    floor of the attention stage; all copies live on DVE except where
    ScalarE is provably idle (chunk boundaries).
  - The inner loop is ACT-bound by ~150-300ns/iteration; any recurring
    PE idle re-throttles the HAM clock gate to 1.2 GHz, which was the
    dominant cost of the baseline (2/3 of the kernel ran cold).  All
    independent PE work (previous chunk's output projection, next
    chunk's q transposes) is therefore distributed through the g-loop
    as single-piece fillers, and the next q-chunk's projection runs at
    the chunk boundary with its bias-adds on the then-idle ScalarE.
  - Normalization packs both heads' numerators/denominators into single
    [128, 512] tiles (exact reciprocal cost scales with free size only;
    the custom-ucode approx reciprocals return garbage on this runtime),
    and the psum accumulators are freed by cheap copies before the
    reciprocal chain runs.
  - The final normalize+output-projection run in q-column halves so the
    tail overlaps the last reciprocal; activation loads are split in
    c-halves so the first transposes start after half a chunk lands.
"""

import numpy as np

B, T, C, H, D = 2, 2048, 1024, 16, 64
NCORES = 8
GROUPS = 4              # head-groups == cores per batch element
HG = H // GROUPS        # heads per core
DS = HG * D             # per-core projection slice width (256)
TCH = 512               # token chunk (psum bank = 512 fp32)
NTCH = T // TCH         # 4
NCC = C // 128          # 8 contraction chunks
NKT = T // 128          # 16 key tiles
SCALE = float(D) ** -0.5

_NC_CACHE = None

# debug: when True, dump intermediates (xt tile, VN tile, rec tile) to DRAM
DEBUG_DUMPS = False


def _emit(ctx, tc, io):
    from concourse import mybir

    nc = tc.nc
    f32 = mybir.dt.float32
    bf16 = mybir.dt.bfloat16
    EXP = mybir.ActivationFunctionType.Exp

    persist = ctx.enter_context(tc.tile_pool(name="persist", bufs=1))

    def ptile(tag, shape, dt=bf16):
        return persist.tile(shape, dt, tag=tag, name=tag)

    # --- persistent SBUF tensors ---------------------------------------
    # load order matches consumer order (wkt/wvt feed the first
    # projections ~10us in; wqt/wot/bos much later), all on the gpsimd
    # ring so the sync/scalar rings stay clear for activation chunks
    ident = ptile("ident", [128, 128])
    nc.gpsimd.dma_start(ident[:], io["ident"][:, :])

    def load_w(name):
        tiles = []
        for c in range(NCC):
            t_ = ptile(f"{name}{c}", [128, DS])
            nc.gpsimd.dma_start(t_[:], io[name][c * 128:(c + 1) * 128, :])
            tiles.append(t_)
        return tiles

    bias = {}

    def load_b(name, width):
        t_ = ptile(name, [128, width], f32)
        nc.gpsimd.dma_start(
            t_[:], io[name].rearrange("(a p) o -> p (a o)", p=128))
        bias[name] = t_

    wsb = {"wkt": load_w("wkt")}
    load_b("bks", 2)
    wsb["wvt"] = load_w("wvt")
    wsb["wqt"] = load_w("wqt")
    load_b("bqs", 2)
    wot = []
    for dc in range(2):
        t_ = ptile(f"wot{dc}", [128, C])
        nc.gpsimd.dma_start(t_[:], io["wot"][dc * 128:(dc + 1) * 128, :])
        wot.append(t_)
    load_b("bos", 8)

    QT = [ptile(f"qt{i}", [128, T]) for i in range(2)]
    KT = [ptile(f"kt{i}", [128, T]) for i in range(2)]
    # V natural [k, head*(64 V + 64 ones)]: the ones columns make each
    # head's AV matmul also produce the softmax denominator (replicated
    # across psum rows 64-127)
    VN = [ptile(f"vn{i}", [128, 4 * 128]) for i in range(NKT)]
    for i in range(NKT):
        ones_view = VN[i][:].rearrange("p (h c) -> p h c", h=4)[:, :, 64:128]
        nc.gpsimd.memset(ones_view, 1.0)


    # --- unified pipeline: k/v prefix, then per-q-chunk attention with
    # the next q-chunk's projection and the previous chunk's output
    # projection used as PE fillers at the normalize boundaries ----------
    with tc.tile_pool(name="nat", bufs=4) as natp, \
         tc.tile_pool(name="xtsb", bufs=10) as xtsbp, \
         tc.tile_pool(name="xtps", bufs=2, space="PSUM") as xtps, \
         tc.tile_pool(name="fps", bufs=2, space="PSUM") as fps, \
         tc.tile_pool(name="otps", bufs=2, space="PSUM") as otps, \
         tc.tile_pool(name="expsb", bufs=4) as expsb, \
         tc.tile_pool(name="otsb", bufs=4) as otsbp, \
         tc.tile_pool(name="recsb", bufs=4) as recp, \
         tc.tile_pool(name="outsb", bufs=3) as outsbp:

        def load_nat(xname, tci, eng):
            nat = natp.tile([128, 4 * C], bf16, tag="nat", name="nat")
            src_ap = io[xname][tci * TCH:(tci + 1) * TCH, :].rearrange(
                "(j p) c -> p j c", p=128)
            dst_ap = nat[:].rearrange("p (j c) -> p j c", j=4)
            # two c-halves: the first 4 transpose groups only depend on
            # the first half (deps are range-tracked), halving the
            # DMA-to-first-transpose latency
            eng.dma_start(dst_ap[:, :, 0:C // 2], src_ap[:, :, 0:C // 2])
            eng.dma_start(dst_ap[:, :, C // 2:C], src_ap[:, :, C // 2:C])
            return nat

        def transpose(nat, copy_engs):
            """PE-transpose a loaded chunk to 8 channel-major [128, 512]
            bf16 tiles; psum->sbuf copies round-robin over copy_engs."""
            xts = []
            for c in range(NCC):
                ps = xtps.tile([128, TCH], bf16, tag="xt", name="xtps")
                for j in range(4):
                    nc.tensor.matmul(
                        ps[:, j * 128:(j + 1) * 128],
                        lhsT=nat[:, j * C + c * 128:j * C + (c + 1) * 128],
                        rhs=ident[:],
                        is_transpose=True,
                        start=(j == 0), stop=(j == 3))
                xt = xtsbp.tile([128, TCH], bf16, tag="xt", name="xtsb")
                copy_engs[c % len(copy_engs)](xt[:], ps[:])
                xts.append(xt)
            return xts

        def proj_qk_co(xts, wname, bname, XT, tci, co, on_scalar=False):
            pj = fps.tile([128, TCH], f32, tag="s", name="proj")
            for c in range(NCC):
                nc.tensor.matmul(
                    pj[:],
                    lhsT=wsb[wname][c][:, co * 128:(co + 1) * 128],
                    rhs=xts[c][:],
                    start=(c == 0), stop=(c == NCC - 1))
            dst = XT[co][:, tci * TCH:(tci + 1) * TCH]
            if on_scalar:
                # at the pr1 boundary ScalarE has no pending exps, while
                # the DVE queue holds the whole normalize chain; the next
                # chunk's first scores serialize on this add via the psum
                # ring, so keep it off DVE
                nc.scalar.activation(
                    dst, pj[:], mybir.ActivationFunctionType.Identity,
                    bias=bias[bname][:, co:co + 1])
            else:
                nc.vector.tensor_scalar_add(
                    dst, pj[:], bias[bname][:, co:co + 1])

        def proj_qk(xts, wname, bname, XT, tci, on_scalar=False):
            for co in range(2):
                proj_qk_co(xts, wname, bname, XT, tci, co, on_scalar)

        def proj_v(xts, tci):
            # swap operands: v^T tile stationary, Wv^T moving -> V natural
            for u in range(4):
                pv = fps.tile([128, TCH], f32, tag="s", name="pv")
                for c in range(NCC):
                    nc.tensor.matmul(
                        pv[:, 0:DS],
                        lhsT=xts[c][:, u * 128:(u + 1) * 128],
                        rhs=wsb["wvt"][c][:],
                        start=(c == 0), stop=(c == NCC - 1))
                vn = VN[tci * 4 + u]
                dst = vn[:].rearrange("p (h c) -> p h c", h=4)[:, :, 0:64]
                src = pv[:, 0:DS].rearrange("p (h d) -> p h d", h=4)
                nc.vector.tensor_copy(dst, src)

        # k/v prefix: both chunks' transposes precede either projection
        # (the projections wait on the weight DMAs early on; the
        # transposes only need the activation chunks, so they keep the
        # PE warm meanwhile)
        nats = {}
        nats["k0"] = load_nat("xk", 0, nc.sync)
        nats["v0"] = load_nat("xv", 0, nc.scalar)
        for tci in range(NTCH):
            if tci + 1 < NTCH:
                nats[f"k{tci + 1}"] = load_nat("xk", tci + 1, nc.sync)
                nats[f"v{tci + 1}"] = load_nat("xv", tci + 1, nc.scalar)
            xts_k = transpose(nats.pop(f"k{tci}"), [nc.vector.tensor_copy])
            xts_v = transpose(nats.pop(f"v{tci}"), [nc.vector.tensor_copy])
            proj_qk(xts_k, "wkt", "bks", KT, tci)
            proj_v(xts_v, tci)
        # q chunk 0 must precede attention; later q chunks are projected
        # inside the attention loop as PE filler
        natq = load_nat("xq", 0, nc.sync)
        xts = transpose(natq, [nc.vector.tensor_copy])
        proj_qk(xts, "wqt", "bqs", QT, 0)

        ot_sb = {}          # qc -> [osb_pr0, osb_pr1]

        def outproj_piece(qc, ct, tail=False, half=None):
            w = TCH if half is None else TCH // 2
            q0 = qc * TCH + (0 if half in (None, 0) else TCH // 2)
            qcols = slice(q0, q0 + w)
            rcols = slice(q0 - qc * TCH, q0 - qc * TCH + w)
            pp = fps.tile([128, TCH], f32, tag="s", name="prj")
            for dc in range(2):
                nc.tensor.matmul(
                    pp[:, 0:w],
                    lhsT=wot[dc][:, ct * 128:(ct + 1) * 128],
                    rhs=ot_sb[qc][dc][:, rcols],
                    start=(dc == 0), stop=(dc == 1))
            ob = outsbp.tile([128, TCH], bf16, tag="ob", name="ob")
            if tail and ct % 2:
                nc.scalar.activation(
                    ob[:, 0:w], pp[:, 0:w],
                    mybir.ActivationFunctionType.Identity,
                    bias=bias["bos"][:, ct:ct + 1])
            else:
                nc.vector.tensor_scalar_add(
                    ob[:, 0:w], pp[:, 0:w], bias["bos"][:, ct:ct + 1])
            dma_eng = (nc.sync if (tail and ct % 2) else nc.gpsimd)
            dma_eng.dma_start(
                io["out_t"][ct * 128:(ct + 1) * 128, qcols], ob[:, 0:w])

        def qtrans_piece(natq, xts_out, c):
            ps = xtps.tile([128, TCH], bf16, tag="xt", name="xtps")
            for j in range(4):
                nc.tensor.matmul(
                    ps[:, j * 128:(j + 1) * 128],
                    lhsT=natq[:, j * C + c * 128:j * C + (c + 1) * 128],
                    rhs=ident[:],
                    is_transpose=True,
                    start=(j == 0), stop=(j == 3))
            xt = xtsbp.tile([128, TCH], bf16, tag="xt", name="xtsb")
            if c < 2:
                # the first pieces run right at the chunk boundary where
                # the DVE queue still holds the normalize chain; ScalarE
                # is idle there
                nc.scalar.copy(xt[:], ps[:])
            else:
                nc.vector.tensor_copy(xt[:], ps[:])
            xts_out.append(xt)

        natq_next = None
        for qc in range(NTCH):
            qcols = slice(qc * TCH, (qc + 1) * TCH)
            ot_sb[qc] = []
            if qc + 1 < NTCH:
                # prefetch the next q chunk early; its transposes are
                # distributed through this chunk's g-loop as PE filler
                natq_next = load_nat("xq", qc + 1, nc.sync)
            # PE filler pieces, one per g iteration: they plug the
            # ~150-300ns/iteration PE idle of the ACT-bound inner loop
            # (which otherwise re-throttles the HAM clock gate to
            # 1.2 GHz) with useful work whose inputs are long ready.
            xts_next = []
            fillers = []
            if qc + 1 < NTCH:
                fillers += [lambda c=c: qtrans_piece(natq_next, xts_next, c)
                            for c in range(NCC)]
            if qc > 0:
                fillers += [lambda ct=ct: outproj_piece(qc - 1, ct)
                            for ct in range(NCC)]
            for pr in range(2):
                otp = [otps.tile([128, TCH], f32, tag="ot", name="ot")
                       for _ in range(2)]
                for g in range(NKT // 2):
                    first = (g == 0)
                    last = (g == NKT // 2 - 1)
                    for hh in range(2):
                        h = pr * 2 + hh
                        rows = slice(hh * 64, (hh + 1) * 64)
                        S = fps.tile([128, 2 * TCH], f32, tag="s", name="s")
                        for j in range(2):
                            kt = 2 * g + j
                            nc.tensor.matmul(
                                S[:, j * TCH:(j + 1) * TCH],
                                lhsT=KT[pr][rows, kt * 128:(kt + 1) * 128],
                                rhs=QT[pr][rows, qcols],
                                start=True, stop=True)
                        es = expsb.tile([128, 2 * TCH], bf16, tag="es",
                                        name="es")
                        nc.scalar.activation(es[:], S[:], EXP, scale=SCALE)
                        for j in range(2):
                            kt = 2 * g + j
                            nc.tensor.matmul(
                                otp[hh][:, :],
                                lhsT=VN[kt][:, h * 128:(h + 1) * 128],
                                rhs=es[:, j * TCH:(j + 1) * TCH],
                                start=(first and j == 0),
                                stop=(last and j == 1))
                    if fillers:
                        fillers.pop(0)()
                # copy the accumulators out of PSUM immediately (the bank
                # frees in ~0.4us instead of after the 4us reciprocal
                # chain), then normalize from SBUF
                if pr == 1 and qc + 1 < NTCH:
                    # next q-chunk projection first: its DVE bias-adds
                    # must precede the reciprocal in the DVE queue, or
                    # the next chunk's scores stall ~3us on QT
                    proj_qk(xts_next, "wqt", "bqs", QT, qc + 1,
                            on_scalar=True)
                # pack both heads' numerators into one [128, x] tile and
                # both denominators into another: reciprocal cost scales
                # with free size only, so one exact reciprocal (the
                # custom-ucode approx variants return garbage on this
                # runtime) and one multiply normalize both heads at once,
                # and the psum banks free after cheap copies
                final = (pr == 1 and qc == NTCH - 1)
                otcv = recp.tile([128, TCH], f32, tag="otc", name="otcv")
                otcd = recp.tile([128, TCH], f32, tag="otd", name="otcd")
                for hh in range(2):
                    rows = slice(hh * 64, (hh + 1) * 64)
                    nc.vector.tensor_copy(otcv[rows, :], otp[hh][0:64, :])
                    nc.vector.tensor_copy(otcd[rows, :], otp[hh][64:128, :])
                osb = otsbp.tile([128, TCH], bf16, tag="otsb", name="otsb")
                ot_sb[qc].append(osb)
                if not final:
                    rec = recp.tile([128, TCH], f32, tag="rec", name="rec")
                    nc.vector.reciprocal(rec[:], otcd[:])
                    nc.vector.tensor_mul(osb[:], otcv[:], rec[:])
                    if pr == 1:
                        for fill in fillers:
                            fill()
                        del fillers[:]
                else:
                    # last normalize: q-column halves so the tail
                    # out-projection overlaps the reciprocal latency
                    for half in range(2):
                        hcols = slice(half * (TCH // 2),
                                      (half + 1) * (TCH // 2))
                        rech = recp.tile([128, TCH // 2], f32, tag="rech",
                                         name="rech")
                        nc.vector.reciprocal(rech[:], otcd[:, hcols])
                        nc.vector.tensor_mul(osb[:, hcols],
                                             otcv[:, hcols], rech[:])
                        for ct in range(NCC):
                            outproj_piece(qc, ct, tail=True, half=half)



def build_nc(reps=1):
    from contextlib import ExitStack

    import concourse.tile as tile
    from concourse import bacc, mybir

    f32 = mybir.dt.float32
    bf16 = mybir.dt.bfloat16
    nc = bacc.Bacc("TRN2", target_bir_lowering=False, debug=False,
                   num_devices=NCORES)
    io = {}
    for name in ("xq", "xk", "xv"):
        io[name] = nc.dram_tensor(name, [T, C], bf16,
                                  kind="ExternalInput").ap()
    for name in ("wqt", "wkt", "wvt"):
        io[name] = nc.dram_tensor(name, [C, DS], bf16,
                                  kind="ExternalInput").ap()
    io["wot"] = nc.dram_tensor("wot", [DS, C], bf16,
                               kind="ExternalInput").ap()
    for name in ("bqs", "bks"):
        io[name] = nc.dram_tensor(name, [DS, 1], f32,
                                  kind="ExternalInput").ap()
    io["bos"] = nc.dram_tensor("bos", [C, 1], f32, kind="ExternalInput").ap()
    io["ident"] = nc.dram_tensor("ident", [128, 128], bf16,
                                 kind="ExternalInput").ap()
    io["out_t"] = nc.dram_tensor("out_t", [C, T], bf16,
                                 kind="ExternalOutput").ap()
    if DEBUG_DUMPS:
        io["dbg_xt"] = nc.dram_tensor("dbg_xt", [128, TCH], bf16,
                                      kind="ExternalOutput").ap()
        io["dbg_vn"] = nc.dram_tensor("dbg_vn", [128, 512], bf16,
                                      kind="ExternalOutput").ap()
        io["dbg_es"] = nc.dram_tensor("dbg_es", [128, 2 * TCH], bf16,
                                      kind="ExternalOutput").ap()
        io["dbg_rec"] = nc.dram_tensor("dbg_rec", [64, TCH], f32,
                                       kind="ExternalOutput").ap()

    with tile.TileContext(nc) as tc:
        if reps == 1:
            with ExitStack() as ctx:
                _emit(ctx, tc, io)
        else:
            with tc.For_i(0, reps, 1):
                with ExitStack() as ctx:
                    _emit(ctx, tc, io)
    nc.compile()
    return nc


def get_nc():
    global _NC_CACHE
    if _NC_CACHE is None:
        _NC_CACHE = build_nc()
    return _NC_CACHE


def make_in_maps(q, k, v, Wq, bq, Wk, bk, Wv, bv, Wo, bo):
    import ml_dtypes

    bf16 = ml_dtypes.bfloat16
    q, k, v = (np.asarray(x, np.float32) for x in (q, k, v))
    Wq, Wk, Wv, Wo = (np.asarray(x, np.float32) for x in (Wq, Wk, Wv, Wo))
    bq, bk, bv, bo = (np.asarray(x, np.float32) for x in (bq, bk, bv, bo))
    xb = [np.ascontiguousarray(t[b]).astype(bf16)
          for t in (q, k, v) for b in range(B)]
    xq_b, xk_b, xv_b = xb[0:2], xb[2:4], xb[4:6]
    ident = np.eye(128, dtype=bf16)
    in_maps = []
    for core in range(NCORES):
        b, g = divmod(core, GROUPS)
        sl = slice(g * DS, (g + 1) * DS)
        # bv enters before the (linear) output projection, so its
        # contribution Wo[:, sl] @ bv[sl] folds into the output bias; bo
        # itself rides on the g == 0 core only.
        bos = Wo[:, sl] @ bv[sl]
        if g == 0:
            bos = bos + bo
        in_maps.append({
            "xq": xq_b[b],
            "xk": xk_b[b],
            "xv": xv_b[b],
            "wqt": np.ascontiguousarray(Wq[sl, :].T).astype(bf16),
            "wkt": np.ascontiguousarray(Wk[sl, :].T).astype(bf16),
            "wvt": np.ascontiguousarray(Wv[sl, :].T).astype(bf16),
            "wot": np.ascontiguousarray(Wo[:, sl].T).astype(bf16),
            "bqs": np.ascontiguousarray(bq[sl].reshape(DS, 1)),
            "bks": np.ascontiguousarray(bk[sl].reshape(DS, 1)),
            "bos": np.ascontiguousarray(bos.reshape(C, 1), np.float32),
            "ident": ident,
        })
    return in_maps


def combine(results):
    out = np.zeros((B, T, C), np.float32)
    for core in range(NCORES):
        b, _ = divmod(core, GROUPS)
        out[b] += results[core]["out_t"].astype(np.float32).T
    return out


def kernel(q, k, v, Wq, bq, Wk, bk, Wv, bv, Wo, bo):
    from concourse.bass_utils import run_bass_kernel_spmd

    nc = get_nc()
    in_maps = make_in_maps(q, k, v, Wq, bq, Wk, bk, Wv, bv, Wo, bo)
    res = run_bass_kernel_spmd(nc, in_maps, core_ids=list(range(NCORES)))
    return combine(res.results)
